# revision 39
# baseline (speedup 1.0000x reference)
"""Sparse-attention Trainium2 kernel (8 NeuronCores, sequence-parallel v2).

Problem (hardcoded): B=1, S=4096, H=1024, NH=16, D=64, K=32.

Sharding (v2): fully sequence-parallel. Core c owns query rows
[512c, 512c+512). It computes q/k/v for its own rows against the FULL
weight matrices (1/8 of the total FLOPs, no communication), publishes its
k|v rows via an 8-way AllGather (bf16, 4 KB/row), then gathers per-query
k/v rows for ALL 16 heads at once (4 KB/descriptor -> 8x fewer SWDGE
descriptors than head-parallel), computes sparse attention for its rows,
and the o-projection rows. Host concatenates row slices and adds bo.

Per-tile layout (16 queries/tile, 32 tiles/core): gather position
pos = j*16 + q -> SBUF partition p = 16*(j%8) + q, chunk cc = j//8.
So partition p holds query tb+p%16, slot block b=p//16; slot j = cc*8+b.
Cross-partition sums over the 8 slot-blocks (softmax denominator, AV
accumulation) are TensorE matmuls against a static 0/1 selection matrix
S16[p, m] = (p%16 == m); q/denominator replication back to 128
partitions uses S16^T. Softmax skips max-subtraction (logits here are
~N(0, 0.42); exp is far from overflow).
"""

import os
from contextlib import ExitStack

import numpy as np
import ml_dtypes

S, H, NH, D, K = 4096, 1024, 16, 64, 32
NCORES = 8
SC = S // NCORES            # 512 rows per core
QT = 16                     # queries per attention tile
NTB = SC // QT              # 32 attention tiles per core
NST = SC // 128             # 4 projection s-tiles per core
CH = NH * D                 # 1024 kv channels per tensor
ROW = 2 * CH                # 2048 bf16 elems per kv row (4 KB)
NCC = K // 8                # 4 slot chunks per tile
BF16 = ml_dtypes.bfloat16

_nc_cache = None


def build_nc(mode="full"):
    import concourse.bass as bass
    import concourse.mybir as mybir
    import concourse.tile as tile
    from concourse import bacc
    from concourse.tile_rust import add_dep_helper
    from concourse.bass import ts, ds

    dt = mybir.dt
    nc = bacc.Bacc("TRN2", target_bir_lowering=False, debug=False,
                   num_devices=NCORES)

    xT = nc.dram_tensor("xT", [H, SC], dt.bfloat16, kind="ExternalInput")
    w3T = nc.dram_tensor("w3T", [H, 3 * CH], dt.bfloat16, kind="ExternalInput")
    woT = nc.dram_tensor("woT", [CH, H], dt.bfloat16, kind="ExternalInput")
    gb = nc.dram_tensor("gb", [128, NTB * 4 * NH], dt.float32, kind="ExternalInput")
    idx16 = nc.dram_tensor("idx16", [128, NTB * (QT * K // 16)], dt.int16,
                           kind="ExternalInput")
    s16d = nc.dram_tensor("s16", [128, 16], dt.bfloat16, kind="ExternalInput")
    qidxd = nc.dram_tensor("qidx16", [128, NTB * 8], dt.int16, kind="ExternalInput")
    identd = nc.dram_tensor("ident", [16, 16], dt.bfloat16, kind="ExternalInput")
    outd = nc.dram_tensor("out", [SC, H], dt.float32, kind="ExternalOutput")
    kv_loc = nc.dram_tensor("kv_loc", [SC, ROW], dt.bfloat16, kind="Internal")
    q_dram = nc.dram_tensor("q_dram", [SC, CH], dt.bfloat16, kind="Internal")
    kv_full = nc.dram_tensor("kv_full", [S, ROW], dt.bfloat16, kind="Internal",
                             addr_space="Shared")

    EXP = mybir.ActivationFunctionType.Exp
    ADD = mybir.AluOpType.add
    X = mybir.AxisListType.X

    with ExitStack() as ctx:
        tc = ctx.enter_context(tile.TileContext(nc))
        const = ctx.enter_context(tc.tile_pool(name="const", bufs=1))

        # ---- resident tensors ----
        w3_sb = const.tile([128, 8, 3 * CH], dt.bfloat16)     # 6 MB
        for kc in range(8):
            nc.sync.dma_start(w3_sb[:, kc, :], w3T[ts(kc, 128), :])
        wo_sb = const.tile([128, 8, H], dt.bfloat16)          # 2 MB
        for ch in range(8):
            nc.sync.dma_start(wo_sb[:, ch, :], woT[ts(ch, 128), :])
        xT_sb = const.tile([128, 8, SC], dt.bfloat16)         # 1 MB
        for kc in range(8):
            nc.sync.dma_start(xT_sb[:, kc, :], xT[ts(kc, 128), :])
        gb_sb = const.tile([128, NTB, 4 * NH], dt.float32)    # 1 MB
        nc.sync.dma_start(gb_sb[:], gb[:, :])
        idx_sb = const.tile([128, NTB * 32], dt.int16)        # 0.25 MB
        nc.sync.dma_start(idx_sb[:], idx16[:, :])
        s16_sb = const.tile([128, 16], dt.bfloat16)
        nc.sync.dma_start(s16_sb[:], s16d[:, :])
        qidx_sb = const.tile([128, NTB * 8], dt.int16)
        nc.sync.dma_start(qidx_sb[:], qidxd[:, :])
        ident_sb = const.tile([16, 16], dt.bfloat16)
        nc.sync.dma_start(ident_sb[:], identd[:, :])

        # ---- pools ----
        kv_pool = ctx.enter_context(tc.tile_pool(name="kvout", bufs=2))
        ps_big = ctx.enter_context(tc.tile_pool(name="ps_big", bufs=3, space="PSUM"))
        ps_sm = ctx.enter_context(tc.tile_pool(name="ps_sm", bufs=2, space="PSUM"))
        gat = ctx.enter_context(tc.tile_pool(name="gat", bufs=2))
        big = ctx.enter_context(tc.tile_pool(name="big", bufs=2))
        small = ctx.enter_context(tc.tile_pool(name="small", bufs=3))
        atg_pool = ctx.enter_context(tc.tile_pool(name="atg", bufs=2))
        outp = ctx.enter_context(tc.tile_pool(name="outp", bufs=2))

        # ---- phase A: k/v first (AllGather can start early), then q ----
        kv_stores = []
        q_stores = []
        for st in range(NST):
            kvt_cur = None
            for pj in (1, 2, 0):      # k, v, q
                ps = ps_big.tile([128, CH], dt.float32, tag="psb")
                for n in range(2):
                    for kc in range(8):
                        nc.tensor.matmul(
                            ps[:, ts(n, 512)],
                            xT_sb[:, kc, ts(st, 128)],
                            w3_sb[:, kc, ds(pj * CH + n * 512, 512)],
                            start=(kc == 0), stop=(kc == 7))
                if pj == 0:
                    qt = kv_pool.tile([128, CH], dt.bfloat16, tag="qt")
                    nc.scalar.copy(qt[:], ps[:])
                    q_stores.append(nc.sync.dma_start(q_dram[ts(st, 128), :], qt[:]))
                elif pj == 1:
                    kvt_cur = kv_pool.tile([128, 2, CH], dt.bfloat16, tag="kvt")
                    nc.scalar.copy(kvt_cur[:, 0, :], ps[:])
                else:
                    nc.scalar.copy(kvt_cur[:, 1, :], ps[:])
                    stn = nc.sync.dma_start(
                        kv_loc[ts(st, 128), :],
                        kvt_cur[:].rearrange("p a b -> p (a b)"))
                    kv_stores.append(stn)

        # ---- kv AllGather ----
        cc_i = nc.gpsimd.collective_compute(
            "AllGather", mybir.AluOpType.bypass,
            replica_groups=[list(range(NCORES))],
            ins=[kv_loc[:, :]], outs=[kv_full[:, :]])
        for stn in kv_stores:
            add_dep_helper(cc_i.ins, stn.ins, sync=True, reason="cc after kv stores")

        # ---- phase B: per-tile sparse attention ----
        atg_cur = None
        for t in range(NTB if mode != "proj" else 0):
            st, g16 = t // 8, t % 8
            # 1. gather k/v rows: [128, NCC, ROW]
            kvsel = gat.tile([128, NCC, ROW], dt.bfloat16, tag="kvsel")
            g = nc.gpsimd.dma_gather(
                out_ap=kvsel[:], in_ap=kv_full[:, :],
                idxs_ap=idx_sb[:, ds(t * 32, 32)],
                num_idxs=QT * K, num_idxs_reg=QT * K,
                elem_size=ROW, single_packet=False)
            add_dep_helper(g.ins, cc_i.ins, sync=True, reason="gather after cc")

            # 2. q replicated to 128 partitions (q[p%16]) via tiny row-gather
            qrep = small.tile([128, 1, CH], dt.bfloat16, tag="qrep")
            gq = nc.gpsimd.dma_gather(
                out_ap=qrep[:], in_ap=q_dram[:, :],
                idxs_ap=qidx_sb[:, ds(t * 8, 8)],
                num_idxs=128, num_idxs_reg=128,
                elem_size=CH, single_packet=False)
            add_dep_helper(gq.ins, q_stores[st].ins, sync=True,
                           reason="qrep gather after q store")

            # 3. t1 = qrep (bcast over chunks) * k_sel
            t1 = big.tile([128, NCC, CH], dt.bfloat16, tag="t1")
            k_ap = kvsel[:, :, 0:CH]
            k_ap2, q_ap2 = bass.broadcast_tensor_aps(k_ap, qrep[:, 0:1, :])
            nc.vector.tensor_mul(t1[:], k_ap2, q_ap2)

            # 4. logits[p, (cc,h)] = sum_d t1 (tree: add halves, reduce 32)
            th = small.tile([128, 4 * NH, 32], dt.bfloat16, tag="th")
            t1v = t1[:].rearrange("p c (h d) -> p (c h) d", d=D)
            nc.vector.tensor_add(th[:], t1v[:, :, 0:32], t1v[:, :, 32:64])
            lgt = small.tile([128, 4 * NH], dt.float32, tag="lgt")
            nc.vector.tensor_reduce(lgt[:], th[:], axis=X, op=ADD)
            nc.vector.tensor_add(lgt[:], lgt[:], gb_sb[:, t, :])

            # 5. e = exp(logits)  (no max subtraction; logits are small)
            e = small.tile([128, NCC, NH, 1], dt.bfloat16, tag="e")
            nc.scalar.activation(e[:].rearrange("p c h o -> p (c h o)"), lgt[:], EXP)

            # 6. denominator: den[q, h] = sum_{b,cc} e  via S16 matmul
            psd = ps_sm.tile([16, NH], dt.float32, tag="pss")
            for cc in range(NCC):
                nc.tensor.matmul(psd[:], s16_sb[:], e[:, cc, :, 0],
                                 start=(cc == 0), stop=(cc == NCC - 1))
            r16 = small.tile([16, NH], dt.float32, tag="r16")
            nc.vector.reciprocal(r16[:], psd[:])
            r16b = small.tile([16, 1, NH], dt.bfloat16, tag="r16b")
            nc.vector.tensor_copy(r16b[:, 0, :], r16[:])

            # 7. W = v_sel * e (bcast over d); normalization happens post-AV.
            #    chunks 0..1 on VectorE, 2..3 on GpSimd (engine balance)
            W = big.tile([128, NCC, CH], dt.bfloat16, tag="W")
            for eng, c0, c1 in ((nc.vector, 0, 2), (nc.gpsimd, 2, NCC)):
                v_ap2, e_ap2 = bass.broadcast_tensor_aps(
                    kvsel[:, c0:c1, CH:ROW].rearrange("p c (h d) -> p c h d", d=D),
                    e[:, c0:c1, :, 0:1])
                eng.tensor_mul(
                    W[:, c0:c1, :].rearrange("p c (h d) -> p c h d", d=D),
                    v_ap2, e_ap2)

            # 8. A[q, hd] = sum_{b,cc} W  via S16 matmul (PSUM accumulate)
            psA = ps_big.tile([16, CH], dt.float32, tag="psb")
            for n in range(2):
                for cc in range(NCC):
                    nc.tensor.matmul(psA[:, ts(n, 512)], s16_sb[:],
                                     W[:, cc, ts(n, 512)],
                                     start=(cc == 0), stop=(cc == NCC - 1))
            A_raw = small.tile([16, CH], dt.bfloat16, tag="A_raw")
            nc.scalar.copy(A_raw[:], psA[:])

            # 9. normalize: A = A_raw * (1/den) (bcast over d)
            A_sb = small.tile([16, CH], dt.bfloat16, tag="A_sb")
            a_in, r_in = bass.broadcast_tensor_aps(
                A_raw[:].rearrange("p (h d) -> p h d", d=D),
                r16b[:, 0:1, :].rearrange("p o h -> p h o"))
            nc.vector.tensor_mul(A_sb[:].rearrange("p (h d) -> p h d", d=D),
                                 a_in, r_in)

            # 11. A^T chunks via PE transpose -> group buffer [128, 8, 128]
            if g16 == 0:
                atg_cur = atg_pool.tile([128, 8, 128], dt.bfloat16, tag="atg")
            psT = ps_sm.tile([128, 8, QT], dt.bfloat16, tag="pss")
            for chk in range(8):
                nc.tensor.transpose(psT[:, chk, :], A_sb[:, ts(chk, 128)],
                                    ident_sb[:])
            nc.vector.tensor_copy(atg_cur[:, :, ds(QT * g16, QT)], psT[:])

            # 12. o-proj per group of 8 tiles (128 query rows)
            if g16 == 7:
                psP = ps_big.tile([128, H], dt.float32, tag="psb")
                for n in range(2):
                    for chk in range(8):
                        nc.tensor.matmul(psP[:, ts(n, 512)], atg_cur[:, chk, :],
                                         wo_sb[:, chk, ts(n, 512)],
                                         start=(chk == 0), stop=(chk == 7))
                ot = outp.tile([128, H], dt.float32, tag="ot")
                nc.scalar.copy(ot[:], psP[:])
                nc.sync.dma_start(outd[ts(st, 128), :], ot[:])

    nc.compile()
    return nc


def prep_inputs(x, idx, valid, geo_bias, Wq, Wk, Wv, Wo, bo):
    """Host-side shard prep. Returns (in_maps, bo_f32)."""
    x = np.asarray(x)
    idx = np.asarray(idx)
    geo_bias = np.asarray(geo_bias)
    Wq, Wk, Wv, Wo = (np.asarray(w) for w in (Wq, Wk, Wv, Wo))
    bo = np.asarray(bo, dtype=np.float32)

    x2 = x.reshape(S, H)
    scale = np.float32(1.0 / np.sqrt(D))
    w3T = np.ascontiguousarray(
        np.concatenate([(Wq * scale).T, Wk.T, Wv.T], axis=1).astype(BF16))
    woT = np.ascontiguousarray(Wo.T.astype(BF16))
    s16 = np.zeros((128, 16), dtype=BF16)
    s16[np.arange(128), np.arange(128) % 16] = 1
    ident = np.eye(16, dtype=BF16)
    # qrep gather: tile t, pos p -> q row t*16 + p%16
    qidx = np.empty((16, NTB * 8), dtype=np.int16)
    for t in range(NTB):
        lin = (t * QT + np.arange(128) % 16).astype(np.int16)
        qidx[:, t * 8:(t + 1) * 8] = lin.reshape(8, 16).T
    qidx = np.ascontiguousarray(np.tile(qidx, (8, 1)))

    in_maps = []
    for c in range(NCORES):
        rb = c * SC
        xTc = np.ascontiguousarray(x2[rb:rb + SC].T.astype(BF16))

        # gather indices: tile t, pos = j*16 + q -> idx[rb + t*16 + q, j]
        idxc = np.empty((16, NTB * 32), dtype=np.int16)
        for t in range(NTB):
            blk = idx[rb + t * QT: rb + (t + 1) * QT, :]      # [16 q, 32 j]
            lin = blk.T.reshape(-1)                            # pos = j*16+q
            idxc[:, t * 32:(t + 1) * 32] = lin.reshape(32, 16).T
        idxc = np.ascontiguousarray(np.tile(idxc, (8, 1)))

        # geo bias: gb[p=(b,qq), t, cc*16+h] = geo_bias[h, rb+t*16+qq, cc*8+b]
        g = geo_bias[:, rb:rb + SC, :]                         # [h, 512, j]
        g2 = g.reshape(NH, NTB, QT, NCC, 8)                    # [h, t, qq, cc, b]
        gbt = g2.transpose(4, 2, 1, 3, 0).reshape(128, NTB * 4 * NH)
        gbt = np.ascontiguousarray(gbt, dtype=np.float32)

        in_maps.append({
            "xT": xTc,
            "w3T": w3T,
            "woT": woT,
            "gb": gbt,
            "idx16": idxc,
            "s16": s16,
            "qidx16": qidx,
            "ident": ident,
        })
    return in_maps, bo


def kernel(x, idx, valid, geo_bias, Wq, Wk, Wv, Wo, bo):
    global _nc_cache
    from concourse.bass_utils import run_bass_kernel_spmd

    if _nc_cache is None:
        _nc_cache = build_nc()
    nc = _nc_cache

    in_maps, bo_f32 = prep_inputs(x, idx, valid, geo_bias, Wq, Wk, Wv, Wo, bo)
    res = run_bass_kernel_spmd(nc, in_maps, core_ids=list(range(NCORES)),
                               trace=bool(int(os.environ.get("KTRACE", "0"))))
    out = np.concatenate([r["out"] for r in res.results], axis=0)
    out = out + bo_f32[None, :]
    if res.exec_time_ns is not None:
        kernel.last_exec_time_ns = res.exec_time_ns
    kernel.last_results = res
    return out.reshape(1, S, H).astype(np.float32)


# revision 42
# speedup vs baseline: 2.0471x; 2.0471x over previous
"""Sparse-attention Trainium2 kernel (8 NeuronCores, sequence-parallel v2).

Problem (hardcoded): B=1, S=4096, H=1024, NH=16, D=64, K=32.

Sharding (v2): fully sequence-parallel. Core c owns query rows
[512c, 512c+512). It computes q/k/v for its own rows against the FULL
weight matrices (1/8 of the total FLOPs, no communication), publishes its
k|v rows via an 8-way AllGather (bf16, 4 KB/row), then gathers per-query
k/v rows for ALL 16 heads at once (4 KB/descriptor -> 8x fewer SWDGE
descriptors than head-parallel), computes sparse attention for its rows,
and the o-projection rows. Host concatenates row slices and adds bo.

Per-tile layout (16 queries/tile, 32 tiles/core): gather position
pos = j*16 + q -> SBUF partition p = 16*(j%8) + q, chunk cc = j//8.
So partition p holds query tb+p%16, slot block b=p//16; slot j = cc*8+b.
Cross-partition sums over the 8 slot-blocks (softmax denominator, AV
accumulation) are TensorE matmuls against a static 0/1 selection matrix
S16[p, m] = (p%16 == m); q/denominator replication back to 128
partitions uses S16^T. Softmax skips max-subtraction (logits here are
~N(0, 0.42); exp is far from overflow).
"""

import os
from contextlib import ExitStack

import numpy as np
import ml_dtypes

S, H, NH, D, K = 4096, 1024, 16, 64, 32
NCORES = 8
SC = S // NCORES            # 512 rows per core
QT = 16                     # queries per attention tile
NTB = SC // QT              # 32 attention tiles per core
NST = SC // 128             # 4 projection s-tiles per core
CH = NH * D                 # 1024 kv channels per tensor
ROW = 2 * CH                # 2048 bf16 elems per kv row (4 KB)
NCC = K // 8                # 4 slot chunks per tile
BF16 = ml_dtypes.bfloat16

_nc_cache = None


def build_nc(mode="full"):
    import concourse.bass as bass
    import concourse.mybir as mybir
    import concourse.tile as tile
    from concourse import bacc
    from concourse.tile_rust import add_dep_helper
    from concourse.bass import ts, ds

    dt = mybir.dt
    nc = bacc.Bacc("TRN2", target_bir_lowering=False, debug=False,
                   num_devices=NCORES)

    xT = nc.dram_tensor("xT", [H, SC], dt.bfloat16, kind="ExternalInput")
    w3T = nc.dram_tensor("w3T", [H, 3 * CH], dt.bfloat16, kind="ExternalInput")
    woT = nc.dram_tensor("woT", [CH, H], dt.bfloat16, kind="ExternalInput")
    gb = nc.dram_tensor("gb", [128, NTB * 4 * NH], dt.float32, kind="ExternalInput")
    idx16 = nc.dram_tensor("idx16", [128, NTB * (QT * K // 16)], dt.int16,
                           kind="ExternalInput")
    s16d = nc.dram_tensor("s16", [128, 16], dt.bfloat16, kind="ExternalInput")
    qidxd = nc.dram_tensor("qidx16", [128, NTB * 8], dt.int16, kind="ExternalInput")
    identd = nc.dram_tensor("ident", [16, 16], dt.bfloat16, kind="ExternalInput")
    outd = nc.dram_tensor("out", [SC, H], dt.float32, kind="ExternalOutput")
    kv_loc = nc.dram_tensor("kv_loc", [SC, ROW], dt.bfloat16, kind="Internal")
    q_dram = nc.dram_tensor("q_dram", [SC, CH], dt.bfloat16, kind="Internal")
    kv_full = nc.dram_tensor("kv_full", [S, ROW], dt.bfloat16, kind="Internal",
                             addr_space="Shared")

    EXP = mybir.ActivationFunctionType.Exp
    ADD = mybir.AluOpType.add
    X = mybir.AxisListType.X

    with ExitStack() as ctx:
        tc = ctx.enter_context(tile.TileContext(nc))
        const = ctx.enter_context(tc.tile_pool(name="const", bufs=1))

        # ---- resident tensors ----
        wo_sb = const.tile([128, 8, H], dt.bfloat16)          # 2 MB
        for ch in range(8):
            nc.sync.dma_start(wo_sb[:, ch, :], woT[ts(ch, 128), :])
        gb_sb = const.tile([128, NTB, 4 * NH], dt.float32)    # 1 MB
        nc.sync.dma_start(gb_sb[:], gb[:, :])
        idx_sb = const.tile([128, NTB * 32], dt.int16)        # 0.25 MB
        nc.sync.dma_start(idx_sb[:], idx16[:, :])
        s16_sb = const.tile([128, 16], dt.bfloat16)
        nc.sync.dma_start(s16_sb[:], s16d[:, :])
        qidx_sb = const.tile([128, NTB * 8], dt.int16)
        nc.sync.dma_start(qidx_sb[:], qidxd[:, :])
        ident_sb = const.tile([16, 16], dt.bfloat16)
        nc.sync.dma_start(ident_sb[:], identd[:, :])

        # ---- pools (phase A; phase-B pools open after wa closes) ----
        kv_pool = ctx.enter_context(tc.tile_pool(name="kvout", bufs=2))
        ps_big = ctx.enter_context(tc.tile_pool(name="ps_big", bufs=3, space="PSUM"))
        ps_sm = ctx.enter_context(tc.tile_pool(name="ps_sm", bufs=2, space="PSUM"))

        # ---- phase A: k/v first (AllGather can start early), then q ----
        kv_stores = []
        q_stores = []
        wa = tc.tile_pool(name="wa", bufs=1)
        wap = wa.__enter__()
        w3_sb = wap.tile([128, 8, 3 * CH], dt.bfloat16)       # 6 MB, phase A only
        for kc in range(8):
            nc.sync.dma_start(w3_sb[:, kc, :], w3T[ts(kc, 128), :])
        xT_sb = wap.tile([128, 8, SC], dt.bfloat16)           # 1 MB, phase A only
        for kc in range(8):
            nc.sync.dma_start(xT_sb[:, kc, :], xT[ts(kc, 128), :])
        for st in range(NST):
            kvt_cur = None
            for pj in (1, 2):         # k then v
                ps = ps_big.tile([128, CH], dt.float32, tag="psb")
                for n in range(2):
                    for kc in range(8):
                        nc.tensor.matmul(
                            ps[:, ts(n, 512)],
                            xT_sb[:, kc, ts(st, 128)],
                            w3_sb[:, kc, ds(pj * CH + n * 512, 512)],
                            start=(kc == 0), stop=(kc == 7))
                if pj == 1:
                    kvt_cur = kv_pool.tile([128, 2, CH], dt.bfloat16, tag="kvt")
                    nc.scalar.copy(kvt_cur[:, 0, :], ps[:])
                else:
                    nc.scalar.copy(kvt_cur[:, 1, :], ps[:])
                    stn = nc.sync.dma_start(
                        kv_loc[ts(st, 128), :],
                        kvt_cur[:].rearrange("p a b -> p (a b)"))
                    kv_stores.append(stn)
        for st in range(NST):         # q after all k/v (overlaps AllGather)
            ps = ps_big.tile([128, CH], dt.float32, tag="psb")
            for n in range(2):
                for kc in range(8):
                    nc.tensor.matmul(
                        ps[:, ts(n, 512)],
                        xT_sb[:, kc, ts(st, 128)],
                        w3_sb[:, kc, ds(n * 512, 512)],
                        start=(kc == 0), stop=(kc == 7))
            qt = kv_pool.tile([128, CH], dt.bfloat16, tag="qt")
            nc.scalar.copy(qt[:], ps[:])
            q_stores.append(nc.sync.dma_start(q_dram[ts(st, 128), :], qt[:]))
        wa.__exit__(None, None, None)
        gat = ctx.enter_context(tc.tile_pool(name="gat", bufs=4))
        big = ctx.enter_context(tc.tile_pool(name="big", bufs=2))
        small = ctx.enter_context(tc.tile_pool(name="small", bufs=4))
        atg_pool = ctx.enter_context(tc.tile_pool(name="atg", bufs=2))
        outp = ctx.enter_context(tc.tile_pool(name="outp", bufs=2))

        # ---- kv AllGather ----
        cc_i = nc.gpsimd.collective_compute(
            "AllGather", mybir.AluOpType.bypass,
            replica_groups=[list(range(NCORES))],
            ins=[kv_loc[:, :]], outs=[kv_full[:, :]])
        for stn in kv_stores:
            add_dep_helper(cc_i.ins, stn.ins, sync=True, reason="cc after kv stores")

        # ---- phase B: per-tile sparse attention ----
        atg_cur = None
        for t in range(NTB if mode != "proj" else 0):
            st, g16 = t // 8, t % 8
            # 1. gather k/v rows: [128, NCC, ROW]
            kvsel = gat.tile([128, NCC, ROW], dt.bfloat16, tag="kvsel")
            g = nc.gpsimd.dma_gather(
                out_ap=kvsel[:], in_ap=kv_full[:, :],
                idxs_ap=idx_sb[:, ds(t * 32, 32)],
                num_idxs=QT * K, num_idxs_reg=QT * K,
                elem_size=ROW, single_packet=False)
            add_dep_helper(g.ins, cc_i.ins, sync=True, reason="gather after cc")

            # 2. q replicated to 128 partitions (q[p%16]) via tiny row-gather
            qrep = small.tile([128, 1, CH], dt.bfloat16, tag="qrep")
            gq = nc.gpsimd.dma_gather(
                out_ap=qrep[:], in_ap=q_dram[:, :],
                idxs_ap=qidx_sb[:, ds(t * 8, 8)],
                num_idxs=128, num_idxs_reg=128,
                elem_size=CH, single_packet=False)
            add_dep_helper(gq.ins, q_stores[st].ins, sync=True,
                           reason="qrep gather after q store")

            # 3. t1 = qrep (bcast over chunks) * k_sel
            t1 = big.tile([128, NCC, CH], dt.bfloat16, tag="t1")
            k_ap = kvsel[:, :, 0:CH]
            k_ap2, q_ap2 = bass.broadcast_tensor_aps(k_ap, qrep[:, 0:1, :])
            nc.vector.tensor_mul(t1[:], k_ap2, q_ap2)

            # 4. logits[p, (cc,h)] = sum_d t1 (tree: add halves, reduce 32)
            th = small.tile([128, 4 * NH, 32], dt.bfloat16, tag="th")
            t1v = t1[:].rearrange("p c (h d) -> p (c h) d", d=D)
            nc.vector.tensor_add(th[:], t1v[:, :, 0:32], t1v[:, :, 32:64])
            lgt = small.tile([128, 4 * NH], dt.float32, tag="lgt")
            nc.vector.tensor_reduce(lgt[:], th[:], axis=X, op=ADD)
            nc.vector.tensor_add(lgt[:], lgt[:], gb_sb[:, t, :])

            # 5. e = exp(logits)  (no max subtraction; logits are small)
            e = small.tile([128, NCC, NH, 1], dt.bfloat16, tag="e")
            nc.scalar.activation(e[:].rearrange("p c h o -> p (c h o)"), lgt[:], EXP)

            # 6. denominator: den[q, h] = sum_{b,cc} e  via S16 matmul
            psd = ps_sm.tile([16, NH], dt.float32, tag="pss")
            for cc in range(NCC):
                nc.tensor.matmul(psd[:], s16_sb[:], e[:, cc, :, 0],
                                 start=(cc == 0), stop=(cc == NCC - 1))
            r16 = small.tile([16, NH], dt.float32, tag="r16")
            nc.vector.reciprocal(r16[:], psd[:])
            r16b = small.tile([16, 1, NH], dt.bfloat16, tag="r16b")
            nc.vector.tensor_copy(r16b[:, 0, :], r16[:])

            # 7. W = v_sel * e (bcast over d); normalization happens post-AV.
            #    chunks 0..1 on VectorE, 2..3 on GpSimd (engine balance)
            W = big.tile([128, NCC, CH], dt.bfloat16, tag="W")
            v_ap2, e_ap2 = bass.broadcast_tensor_aps(
                kvsel[:, :, CH:ROW].rearrange("p c (h d) -> p c h d", d=D),
                e[:, :, :, 0:1])
            nc.vector.tensor_mul(
                W[:].rearrange("p c (h d) -> p c h d", d=D), v_ap2, e_ap2)

            # 8. A[q, hd] = sum_{b,cc} W  via S16 matmul (PSUM accumulate)
            psA = ps_big.tile([16, CH], dt.float32, tag="psb")
            for n in range(2):
                for cc in range(NCC):
                    nc.tensor.matmul(psA[:, ts(n, 512)], s16_sb[:],
                                     W[:, cc, ts(n, 512)],
                                     start=(cc == 0), stop=(cc == NCC - 1))
            A_raw = small.tile([16, CH], dt.bfloat16, tag="A_raw")
            nc.scalar.copy(A_raw[:], psA[:])

            # 9. normalize: A = A_raw * (1/den) (bcast over d)
            A_sb = small.tile([16, CH], dt.bfloat16, tag="A_sb")
            a_in, r_in = bass.broadcast_tensor_aps(
                A_raw[:].rearrange("p (h d) -> p h d", d=D),
                r16b[:, 0:1, :].rearrange("p o h -> p h o"))
            nc.vector.tensor_mul(A_sb[:].rearrange("p (h d) -> p h d", d=D),
                                 a_in, r_in)

            # 11. A^T chunks via PE transpose -> group buffer [128, 8, 128]
            if g16 == 0:
                atg_cur = atg_pool.tile([128, 8, 128], dt.bfloat16, tag="atg")
            psT = ps_sm.tile([128, 8, QT], dt.bfloat16, tag="pss")
            for chk in range(8):
                nc.tensor.transpose(psT[:, chk, :], A_sb[:, ts(chk, 128)],
                                    ident_sb[:])
            nc.vector.tensor_copy(atg_cur[:, :, ds(QT * g16, QT)], psT[:])

            # 12. o-proj per group of 8 tiles (128 query rows)
            if g16 == 7:
                psP = ps_big.tile([128, H], dt.float32, tag="psb")
                for n in range(2):
                    for chk in range(8):
                        nc.tensor.matmul(psP[:, ts(n, 512)], atg_cur[:, chk, :],
                                         wo_sb[:, chk, ts(n, 512)],
                                         start=(chk == 0), stop=(chk == 7))
                ot = outp.tile([128, H], dt.float32, tag="ot")
                nc.scalar.copy(ot[:], psP[:])
                nc.sync.dma_start(outd[ts(st, 128), :], ot[:])

    nc.compile()
    return nc


def prep_inputs(x, idx, valid, geo_bias, Wq, Wk, Wv, Wo, bo):
    """Host-side shard prep. Returns (in_maps, bo_f32)."""
    x = np.asarray(x)
    idx = np.asarray(idx)
    geo_bias = np.asarray(geo_bias)
    Wq, Wk, Wv, Wo = (np.asarray(w) for w in (Wq, Wk, Wv, Wo))
    bo = np.asarray(bo, dtype=np.float32)

    x2 = x.reshape(S, H)
    scale = np.float32(1.0 / np.sqrt(D))
    w3T = np.ascontiguousarray(
        np.concatenate([(Wq * scale).T, Wk.T, Wv.T], axis=1).astype(BF16))
    woT = np.ascontiguousarray(Wo.T.astype(BF16))
    s16 = np.zeros((128, 16), dtype=BF16)
    s16[np.arange(128), np.arange(128) % 16] = 1
    ident = np.eye(16, dtype=BF16)
    # qrep gather: tile t, pos p -> q row t*16 + p%16
    qidx = np.empty((16, NTB * 8), dtype=np.int16)
    for t in range(NTB):
        lin = (t * QT + np.arange(128) % 16).astype(np.int16)
        qidx[:, t * 8:(t + 1) * 8] = lin.reshape(8, 16).T
    qidx = np.ascontiguousarray(np.tile(qidx, (8, 1)))

    in_maps = []
    for c in range(NCORES):
        rb = c * SC
        xTc = np.ascontiguousarray(x2[rb:rb + SC].T.astype(BF16))

        # gather indices: tile t, pos = j*16 + q -> idx[rb + t*16 + q, j]
        idxc = np.empty((16, NTB * 32), dtype=np.int16)
        for t in range(NTB):
            blk = idx[rb + t * QT: rb + (t + 1) * QT, :]      # [16 q, 32 j]
            lin = blk.T.reshape(-1)                            # pos = j*16+q
            idxc[:, t * 32:(t + 1) * 32] = lin.reshape(32, 16).T
        idxc = np.ascontiguousarray(np.tile(idxc, (8, 1)))

        # geo bias: gb[p=(b,qq), t, cc*16+h] = geo_bias[h, rb+t*16+qq, cc*8+b]
        g = geo_bias[:, rb:rb + SC, :]                         # [h, 512, j]
        g2 = g.reshape(NH, NTB, QT, NCC, 8)                    # [h, t, qq, cc, b]
        gbt = g2.transpose(4, 2, 1, 3, 0).reshape(128, NTB * 4 * NH)
        gbt = np.ascontiguousarray(gbt, dtype=np.float32)

        in_maps.append({
            "xT": xTc,
            "w3T": w3T,
            "woT": woT,
            "gb": gbt,
            "idx16": idxc,
            "s16": s16,
            "qidx16": qidx,
            "ident": ident,
        })
    return in_maps, bo


def kernel(x, idx, valid, geo_bias, Wq, Wk, Wv, Wo, bo):
    global _nc_cache
    from concourse.bass_utils import run_bass_kernel_spmd

    if _nc_cache is None:
        _nc_cache = build_nc()
    nc = _nc_cache

    in_maps, bo_f32 = prep_inputs(x, idx, valid, geo_bias, Wq, Wk, Wv, Wo, bo)
    res = run_bass_kernel_spmd(nc, in_maps, core_ids=list(range(NCORES)),
                               trace=bool(int(os.environ.get("KTRACE", "0"))))
    out = np.concatenate([r["out"] for r in res.results], axis=0)
    out = out + bo_f32[None, :]
    if res.exec_time_ns is not None:
        kernel.last_exec_time_ns = res.exec_time_ns
    kernel.last_results = res
    return out.reshape(1, S, H).astype(np.float32)


# revision 44
# speedup vs baseline: 2.3451x; 1.1456x over previous
"""Sparse-attention Trainium2 kernel (8 NeuronCores, sequence-parallel v2).

Problem (hardcoded): B=1, S=4096, H=1024, NH=16, D=64, K=32.

Sharding (v2): fully sequence-parallel. Core c owns query rows
[512c, 512c+512). It computes q/k/v for its own rows against the FULL
weight matrices (1/8 of the total FLOPs, no communication), publishes its
k|v rows via an 8-way AllGather (bf16, 4 KB/row), then gathers per-query
k/v rows for ALL 16 heads at once (4 KB/descriptor -> 8x fewer SWDGE
descriptors than head-parallel), computes sparse attention for its rows,
and the o-projection rows. Host concatenates row slices and adds bo.

Per-tile layout (16 queries/tile, 32 tiles/core): gather position
pos = j*16 + q -> SBUF partition p = 16*(j%8) + q, chunk cc = j//8.
So partition p holds query tb+p%16, slot block b=p//16; slot j = cc*8+b.
Cross-partition sums over the 8 slot-blocks (softmax denominator, AV
accumulation) are TensorE matmuls against a static 0/1 selection matrix
S16[p, m] = (p%16 == m); q/denominator replication back to 128
partitions uses S16^T. Softmax skips max-subtraction (logits here are
~N(0, 0.42); exp is far from overflow).
"""

import os
from contextlib import ExitStack

import numpy as np
import ml_dtypes

S, H, NH, D, K = 4096, 1024, 16, 64, 32
NCORES = 8
SC = S // NCORES            # 512 rows per core
QT = 16                     # queries per attention tile
NTB = SC // QT              # 32 attention tiles per core
NST = SC // 128             # 4 projection s-tiles per core
CH = NH * D                 # 1024 kv channels per tensor
ROW = 2 * CH                # 2048 bf16 elems per kv row (4 KB)
NCC = K // 8                # 4 slot chunks per tile
BF16 = ml_dtypes.bfloat16

_nc_cache = None


def build_nc(mode="full"):
    import concourse.bass as bass
    import concourse.mybir as mybir
    import concourse.tile as tile
    from concourse import bacc
    from concourse.tile_rust import add_dep_helper
    from concourse.bass import ts, ds

    dt = mybir.dt
    nc = bacc.Bacc("TRN2", target_bir_lowering=False, debug=False,
                   num_devices=NCORES)

    xT = nc.dram_tensor("xT", [H, SC], dt.bfloat16, kind="ExternalInput")
    w3T = nc.dram_tensor("w3T", [H, 3 * CH], dt.bfloat16, kind="ExternalInput")
    woT = nc.dram_tensor("woT", [CH, H], dt.bfloat16, kind="ExternalInput")
    gb = nc.dram_tensor("gb", [128, NTB * 4 * NH], dt.float32, kind="ExternalInput")
    idx16 = nc.dram_tensor("idx16", [128, NTB * (QT * K // 16)], dt.int16,
                           kind="ExternalInput")
    s16d = nc.dram_tensor("s16", [128, 16], dt.bfloat16, kind="ExternalInput")
    qidxd = nc.dram_tensor("qidx16", [128, NTB * 8], dt.int16, kind="ExternalInput")
    identd = nc.dram_tensor("ident", [16, 16], dt.bfloat16, kind="ExternalInput")
    outd = nc.dram_tensor("out", [SC, H], dt.float32, kind="ExternalOutput")
    kv_loc = nc.dram_tensor("kv_loc", [SC, ROW], dt.bfloat16, kind="Internal")
    q_dram = nc.dram_tensor("q_dram", [SC, CH], dt.bfloat16, kind="Internal")
    kv_full = nc.dram_tensor("kv_full", [S, ROW], dt.bfloat16, kind="Internal",
                             addr_space="Shared")

    EXP = mybir.ActivationFunctionType.Exp
    ADD = mybir.AluOpType.add
    X = mybir.AxisListType.X

    with ExitStack() as ctx:
        tc = ctx.enter_context(tile.TileContext(nc))
        const = ctx.enter_context(tc.tile_pool(name="const", bufs=1))

        # ---- resident tensors ----
        wo_sb = const.tile([128, 8, H], dt.bfloat16)          # 2 MB
        for ch in range(8):
            nc.sync.dma_start(wo_sb[:, ch, :], woT[ts(ch, 128), :])
        gb_sb = const.tile([128, NTB, 4 * NH], dt.float32)    # 1 MB
        nc.sync.dma_start(gb_sb[:], gb[:, :])
        idx_sb = const.tile([128, NTB * 32], dt.int16)        # 0.25 MB
        nc.sync.dma_start(idx_sb[:], idx16[:, :])
        s16_sb = const.tile([128, 16], dt.bfloat16)
        nc.sync.dma_start(s16_sb[:], s16d[:, :])
        qidx_sb = const.tile([128, NTB * 8], dt.int16)
        nc.sync.dma_start(qidx_sb[:], qidxd[:, :])
        ident_sb = const.tile([16, 16], dt.bfloat16)
        nc.sync.dma_start(ident_sb[:], identd[:, :])

        # ---- pools (phase A; phase-B pools open after wa closes) ----
        kv_pool = ctx.enter_context(tc.tile_pool(name="kvout", bufs=2))
        ps_big = ctx.enter_context(tc.tile_pool(name="ps_big", bufs=3, space="PSUM"))
        ps_sm = ctx.enter_context(tc.tile_pool(name="ps_sm", bufs=2, space="PSUM"))

        # ---- phase A: k/v first (AllGather can start early), then q ----
        kv_stores = []
        q_stores = []
        wa = tc.tile_pool(name="wa", bufs=1)
        wap = wa.__enter__()
        w3_sb = wap.tile([128, 8, 3 * CH], dt.bfloat16)       # 6 MB, phase A only
        for kc in range(8):
            nc.sync.dma_start(w3_sb[:, kc, :], w3T[ts(kc, 128), :])
        xT_sb = wap.tile([128, 8, SC], dt.bfloat16)           # 1 MB, phase A only
        for kc in range(8):
            nc.sync.dma_start(xT_sb[:, kc, :], xT[ts(kc, 128), :])
        for st in range(NST):
            kvt_cur = None
            for pj in (1, 2):         # k then v
                ps = ps_big.tile([128, CH], dt.float32, tag="psb")
                for n in range(2):
                    for kc in range(8):
                        nc.tensor.matmul(
                            ps[:, ts(n, 512)],
                            xT_sb[:, kc, ts(st, 128)],
                            w3_sb[:, kc, ds(pj * CH + n * 512, 512)],
                            start=(kc == 0), stop=(kc == 7))
                if pj == 1:
                    kvt_cur = kv_pool.tile([128, 2, CH], dt.bfloat16, tag="kvt")
                    nc.scalar.copy(kvt_cur[:, 0, :], ps[:])
                else:
                    nc.scalar.copy(kvt_cur[:, 1, :], ps[:])
                    stn = nc.sync.dma_start(
                        kv_loc[ts(st, 128), :],
                        kvt_cur[:].rearrange("p a b -> p (a b)"))
                    kv_stores.append(stn)
        for st in range(NST):         # q after all k/v (overlaps AllGather)
            ps = ps_big.tile([128, CH], dt.float32, tag="psb")
            for n in range(2):
                for kc in range(8):
                    nc.tensor.matmul(
                        ps[:, ts(n, 512)],
                        xT_sb[:, kc, ts(st, 128)],
                        w3_sb[:, kc, ds(n * 512, 512)],
                        start=(kc == 0), stop=(kc == 7))
            qt = kv_pool.tile([128, CH], dt.bfloat16, tag="qt")
            nc.scalar.copy(qt[:], ps[:])
            q_stores.append(nc.sync.dma_start(q_dram[ts(st, 128), :], qt[:]))
        wa.__exit__(None, None, None)
        gat = ctx.enter_context(tc.tile_pool(name="gat", bufs=4))
        big = ctx.enter_context(tc.tile_pool(name="big", bufs=2))
        small = ctx.enter_context(tc.tile_pool(name="small", bufs=4))
        atg_pool = ctx.enter_context(tc.tile_pool(name="atg", bufs=2))
        outp = ctx.enter_context(tc.tile_pool(name="outp", bufs=2))

        # ---- kv AllGather ----
        cc_i = nc.gpsimd.collective_compute(
            "AllGather", mybir.AluOpType.bypass,
            replica_groups=[list(range(NCORES))],
            ins=[kv_loc[:, :]], outs=[kv_full[:, :]])
        for stn in kv_stores:
            add_dep_helper(cc_i.ins, stn.ins, sync=True, reason="cc after kv stores")

        # ---- phase B: per-tile sparse attention ----
        atg_cur = None
        for t in range(NTB if mode != "proj" else 0):
            st, g16 = t // 8, t % 8
            # 1. gather k/v rows: [128, NCC, ROW]
            kvsel = gat.tile([128, NCC, ROW], dt.bfloat16, tag="kvsel")
            g = nc.gpsimd.dma_gather(
                out_ap=kvsel[:], in_ap=kv_full[:, :],
                idxs_ap=idx_sb[:, ds(t * 32, 32)],
                num_idxs=QT * K, num_idxs_reg=QT * K,
                elem_size=ROW, single_packet=False)
            add_dep_helper(g.ins, cc_i.ins, sync=True, reason="gather after cc")

            # 2. q replicated to 128 partitions (q[p%16]) via tiny row-gather
            qrep = small.tile([128, 1, CH], dt.bfloat16, tag="qrep")
            gq = nc.gpsimd.dma_gather(
                out_ap=qrep[:], in_ap=q_dram[:, :],
                idxs_ap=qidx_sb[:, ds(t * 8, 8)],
                num_idxs=128, num_idxs_reg=128,
                elem_size=CH, single_packet=False)
            add_dep_helper(gq.ins, q_stores[st].ins, sync=True,
                           reason="qrep gather after q store")

            # 3. t1 = qrep (bcast over chunks) * k_sel
            t1 = big.tile([128, NCC, CH], dt.bfloat16, tag="t1")
            k_ap = kvsel[:, :, 0:CH]
            k_ap2, q_ap2 = bass.broadcast_tensor_aps(k_ap, qrep[:, 0:1, :])
            nc.vector.tensor_mul(t1[:], k_ap2, q_ap2)

            # 4. logits[p, (cc,h)] = sum_d t1 (tree: add halves, reduce 32)
            th = small.tile([128, 4 * NH, 32], dt.bfloat16, tag="th")
            t1v = t1[:].rearrange("p c (h d) -> p (c h) d", d=D)
            nc.vector.tensor_add(th[:], t1v[:, :, 0:32], t1v[:, :, 32:64])
            lgt = small.tile([128, 4 * NH], dt.float32, tag="lgt")
            nc.vector.tensor_reduce(lgt[:], th[:], axis=X, op=ADD)
            nc.vector.tensor_add(lgt[:], lgt[:], gb_sb[:, t, :])

            # 5. e = exp(logits)  (no max subtraction; logits are small)
            e = small.tile([128, NCC, NH, 1], dt.bfloat16, tag="e")
            nc.scalar.activation(e[:].rearrange("p c h o -> p (c h o)"), lgt[:], EXP)

            # 6. denominator: den[q, h] = sum_{b,cc} e  via S16 matmul
            psd = ps_sm.tile([16, NH], dt.float32, tag="pss")
            for cc in range(NCC):
                nc.tensor.matmul(psd[:], s16_sb[:], e[:, cc, :, 0],
                                 start=(cc == 0), stop=(cc == NCC - 1))
            r16 = small.tile([16, NH], dt.float32, tag="r16")
            nc.vector.reciprocal(r16[:], psd[:])
            r16b = small.tile([16, 1, NH], dt.bfloat16, tag="r16b")
            nc.scalar.copy(r16b[:, 0, :], r16[:])

            # 7. W = v_sel * e (bcast over d, pair-expanded so DVE hits 2x)
            e2 = small.tile([128, NCC, NH, 1, 2], dt.bfloat16, tag="e2")
            e2a, e2b = bass.broadcast_tensor_aps(
                e2[:].rearrange("p c h o w -> p c h (o w)"), e[:, :, :, 0:1])
            nc.scalar.copy(e2a, e2b)
            W = big.tile([128, NCC, CH], dt.bfloat16, tag="W")
            v_ap2, e_ap2 = bass.broadcast_tensor_aps(
                kvsel[:, :, CH:ROW].rearrange("p c (h dd w) -> p c h dd w", dd=32, w=2),
                e2[:, :, :, 0:1, :])
            nc.vector.tensor_mul(
                W[:].rearrange("p c (h dd w) -> p c h dd w", dd=32, w=2),
                v_ap2, e_ap2)

            # 8. A[q, hd] = sum_{b,cc} W  via S16 matmul (PSUM accumulate)
            psA = ps_big.tile([16, CH], dt.float32, tag="psb")
            for n in range(2):
                for cc in range(NCC):
                    nc.tensor.matmul(psA[:, ts(n, 512)], s16_sb[:],
                                     W[:, cc, ts(n, 512)],
                                     start=(cc == 0), stop=(cc == NCC - 1))
            A_raw = small.tile([16, CH], dt.bfloat16, tag="A_raw")
            nc.scalar.copy(A_raw[:], psA[:])

            # 9. normalize: A = A_raw * (1/den) (bcast over d, pair-expanded)
            r2 = small.tile([16, NH, 1, 2], dt.bfloat16, tag="r2")
            r2a, r2b = bass.broadcast_tensor_aps(
                r2[:].rearrange("p h o w -> p h (o w)"),
                r16b[:, 0:1, :].rearrange("p o h -> p h o"))
            nc.scalar.copy(r2a, r2b)
            A_sb = small.tile([16, CH], dt.bfloat16, tag="A_sb")
            a_in, r_in = bass.broadcast_tensor_aps(
                A_raw[:].rearrange("p (h dd w) -> p h dd w", dd=32, w=2),
                r2[:, :, 0:1, :])
            nc.vector.tensor_mul(
                A_sb[:].rearrange("p (h dd w) -> p h dd w", dd=32, w=2),
                a_in, r_in)

            # 11. A^T chunks via PE transpose -> group buffer [128, 8, 128]
            if g16 == 0:
                atg_cur = atg_pool.tile([128, 8, 128], dt.bfloat16, tag="atg")
            psT = ps_sm.tile([128, 8, QT], dt.bfloat16, tag="pss")
            for chk in range(8):
                nc.tensor.transpose(psT[:, chk, :], A_sb[:, ts(chk, 128)],
                                    ident_sb[:])
            nc.scalar.copy(atg_cur[:, :, ds(QT * g16, QT)], psT[:])

            # 12. o-proj per group of 8 tiles (128 query rows)
            if g16 == 7:
                psP = ps_big.tile([128, H], dt.float32, tag="psb")
                for n in range(2):
                    for chk in range(8):
                        nc.tensor.matmul(psP[:, ts(n, 512)], atg_cur[:, chk, :],
                                         wo_sb[:, chk, ts(n, 512)],
                                         start=(chk == 0), stop=(chk == 7))
                ot = outp.tile([128, H], dt.float32, tag="ot")
                nc.scalar.copy(ot[:], psP[:])
                nc.sync.dma_start(outd[ts(st, 128), :], ot[:])

    nc.compile()
    return nc


def prep_inputs(x, idx, valid, geo_bias, Wq, Wk, Wv, Wo, bo):
    """Host-side shard prep. Returns (in_maps, bo_f32)."""
    x = np.asarray(x)
    idx = np.asarray(idx)
    geo_bias = np.asarray(geo_bias)
    Wq, Wk, Wv, Wo = (np.asarray(w) for w in (Wq, Wk, Wv, Wo))
    bo = np.asarray(bo, dtype=np.float32)

    x2 = x.reshape(S, H)
    scale = np.float32(1.0 / np.sqrt(D))
    w3T = np.ascontiguousarray(
        np.concatenate([(Wq * scale).T, Wk.T, Wv.T], axis=1).astype(BF16))
    woT = np.ascontiguousarray(Wo.T.astype(BF16))
    s16 = np.zeros((128, 16), dtype=BF16)
    s16[np.arange(128), np.arange(128) % 16] = 1
    ident = np.eye(16, dtype=BF16)
    # qrep gather: tile t, pos p -> q row t*16 + p%16
    qidx = np.empty((16, NTB * 8), dtype=np.int16)
    for t in range(NTB):
        lin = (t * QT + np.arange(128) % 16).astype(np.int16)
        qidx[:, t * 8:(t + 1) * 8] = lin.reshape(8, 16).T
    qidx = np.ascontiguousarray(np.tile(qidx, (8, 1)))

    in_maps = []
    for c in range(NCORES):
        rb = c * SC
        xTc = np.ascontiguousarray(x2[rb:rb + SC].T.astype(BF16))

        # gather indices: tile t, pos = j*16 + q -> idx[rb + t*16 + q, j]
        idxc = np.empty((16, NTB * 32), dtype=np.int16)
        for t in range(NTB):
            blk = idx[rb + t * QT: rb + (t + 1) * QT, :]      # [16 q, 32 j]
            lin = blk.T.reshape(-1)                            # pos = j*16+q
            idxc[:, t * 32:(t + 1) * 32] = lin.reshape(32, 16).T
        idxc = np.ascontiguousarray(np.tile(idxc, (8, 1)))

        # geo bias: gb[p=(b,qq), t, cc*16+h] = geo_bias[h, rb+t*16+qq, cc*8+b]
        g = geo_bias[:, rb:rb + SC, :]                         # [h, 512, j]
        g2 = g.reshape(NH, NTB, QT, NCC, 8)                    # [h, t, qq, cc, b]
        gbt = g2.transpose(4, 2, 1, 3, 0).reshape(128, NTB * 4 * NH)
        gbt = np.ascontiguousarray(gbt, dtype=np.float32)

        in_maps.append({
            "xT": xTc,
            "w3T": w3T,
            "woT": woT,
            "gb": gbt,
            "idx16": idxc,
            "s16": s16,
            "qidx16": qidx,
            "ident": ident,
        })
    return in_maps, bo


def kernel(x, idx, valid, geo_bias, Wq, Wk, Wv, Wo, bo):
    global _nc_cache
    from concourse.bass_utils import run_bass_kernel_spmd

    if _nc_cache is None:
        _nc_cache = build_nc()
    nc = _nc_cache

    in_maps, bo_f32 = prep_inputs(x, idx, valid, geo_bias, Wq, Wk, Wv, Wo, bo)
    res = run_bass_kernel_spmd(nc, in_maps, core_ids=list(range(NCORES)),
                               trace=bool(int(os.environ.get("KTRACE", "0"))))
    out = np.concatenate([r["out"] for r in res.results], axis=0)
    out = out + bo_f32[None, :]
    if res.exec_time_ns is not None:
        kernel.last_exec_time_ns = res.exec_time_ns
    kernel.last_results = res
    return out.reshape(1, S, H).astype(np.float32)
